# revision 1
# baseline (speedup 1.0000x reference)
"""ChebNet (K=2) graph classifier on 8 Trainium2 NeuronCores.

Strategy (graph/data parallel, zero halo):
  - The 50 batched graphs are independent (edges never cross graphs), so
    graphs are assigned whole to cores (6-7 per core).  One SPMD program
    runs on all 8 cores; cores with fewer graphs chew zero blocks.
  - The normalized aggregation  Tx1 = -D^-1/2 A D^-1/2 feat  is computed
    as dense 128x512-blocked matmuls on the PE against per-graph
    adjacency blocks S[s, d] = -dinv[s] * dinv[d] * count(s->d), built
    host-side (structural preprocessing: adjacency + degrees only) and
    streamed from HBM as plain sequential DMAs.  At avg degree 16 the
    dense blocks carry the same HBM traffic as a per-edge gather
    (2000*2000*2B = 8MB vs 32k*256B = 8.2MB per graph) but need no
    descriptor generation (which measures ~7ns/edge on the Q7 SWDGE
    path and dominates any gather-based variant).
  - Everything else stays feature-major on-chip: per-graph feature
    chunks are PE-transposed into node-major stationary tiles, the two
    Chebyshev dense layers run as K=128-split matmuls, max-pool readout
    and the classifier run on-device.  fp16 operands, fp32 PSUM.
"""

import sys

if "/opt/trn_rl_repo" not in sys.path:
    sys.path.insert(0, "/opt/trn_rl_repo")

import numpy as np

# ---------------------------------------------------------------- constants
N = 100_000
E = 1_600_000
B = 50
GSIZE = 2000
D = 128  # IN == HID == 128
NCOUT = 10
NCORES = 8
NG = 7  # graph slots per core (50 = 2*7 + 6*6)
NSLAB = 512  # dst columns per aggregation matmul


def _default_cfg():
    nwin = (GSIZE + 127) // 128
    gstride = nwin * 128
    nslab = 500 if GSIZE % 500 == 0 else GSIZE
    return dict(
        ng=NG,
        gsize=GSIZE,
        nwin=nwin,
        gstride=gstride,
        nslab=nslab,
        nquad=GSIZE // nslab,
    )


# ---------------------------------------------------------------- host prep
def _preprocess(src, dst, cfg, n_nodes, n_graphs):
    """Structural preprocessing: graph->core assignment and per-graph
    scaled dense adjacency blocks."""
    gsize, ng, nwin, gstride = cfg["gsize"], cfg["ng"], cfg["nwin"], cfg["gstride"]

    deg = np.bincount(dst, minlength=n_nodes)
    dinv = (np.clip(deg.astype(np.float64), 1.0, None) ** -0.5).astype(np.float32)

    order = [0, 2, 1, 3, 4, 5, 6, 7]  # extra graphs land on cores 0 and 2
    slots = [[] for _ in range(NCORES)]
    for g in range(n_graphs):
        slots[order[g % NCORES]].append(g)

    # per-graph scaled dense adjacency, fp16, [gstride, gstride]
    g_of_e = dst // gsize
    flat = (src - g_of_e * gsize) * np.int64(gstride) + (dst - g_of_e * gsize)
    sblks = []
    for g in range(n_graphs):
        m = g_of_e == g
        cnt = np.bincount(flat[m], minlength=gstride * gstride).astype(np.float32)
        S = cnt.reshape(gstride, gstride)
        dv = np.zeros(gstride, dtype=np.float32)
        dv[:gsize] = dinv[g * gsize : (g + 1) * gsize]
        S *= -dv[:, None]
        S *= dv[None, :]
        sblks.append(S[:, :gsize].astype(np.float16))
    return dict(slots=slots, sblks=sblks)


# ---------------------------------------------------------------- program
def _build_program(cfg):
    from concourse import bacc, mybir, tile

    ng, nwin, gstride, gsize, nquad = (
        cfg["ng"],
        cfg["nwin"],
        cfg["gstride"],
        cfg["gsize"],
        cfg["nquad"],
    )
    nslab = cfg["nslab"]
    ngg = ng * gstride
    f16 = mybir.dt.float16
    f32 = mybir.dt.float32
    AL = mybir.AluOpType

    nc = bacc.Bacc(None, target_bir_lowering=False)

    xt_in = nc.declare_dram_parameter("XT", [128, ngg], f16, isOutput=False)
    # S blocks: [ng, nquad, nwin, 128, NSLAB] src-chunk-major per dst-slab
    sb_in = nc.declare_dram_parameter(
        "SBLK", [ng * nquad * nwin * 128, nslab], f16, isOutput=False
    )
    w1a_in = nc.declare_dram_parameter("W1A", [128, 128], f16, isOutput=False)
    w1b_in = nc.declare_dram_parameter("W1B", [128, 128], f16, isOutput=False)
    w2a_in = nc.declare_dram_parameter("W2A", [128, 128], f16, isOutput=False)
    w2b_in = nc.declare_dram_parameter("W2B", [128, 128], f16, isOutput=False)
    b1_in = nc.declare_dram_parameter("B1", [128, 1], f32, isOutput=False)
    b2_in = nc.declare_dram_parameter("B2", [128, 1], f32, isOutput=False)
    wc_in = nc.declare_dram_parameter("WC", [128, NCOUT], f16, isOutput=False)
    bc_in = nc.declare_dram_parameter("BC", [1, NCOUT], f16, isOutput=False)
    ones_in = nc.declare_dram_parameter("ONES1", [1, ng], f16, isOutput=False)
    id_in = nc.declare_dram_parameter("IDENT", [128, 128], f16, isOutput=False)
    out_dram = nc.declare_dram_parameter("OUT", [ng, NCOUT], f32, isOutput=True)

    # dense N tiling of the real columns
    ntiles = []
    off = 0
    while off < gsize:
        ln = min(500, gsize - off)
        ntiles.append((off, ln))
        off += ln

    with tile.TileContext(nc) as tc:
        with (
            tc.tile_pool(name="const", bufs=1) as cpool,
            tc.tile_pool(name="big", bufs=1) as bigpool,
            tc.tile_pool(name="work", bufs=2) as wpool,
            tc.tile_pool(name="tx1p", bufs=2) as tx1pool,
            tc.tile_pool(name="stgp", bufs=2) as stgpool,
            tc.tile_pool(name="sblkp", bufs=4) as sbpool,
            tc.tile_pool(name="ptr", bufs=2, space="PSUM") as ptrpool,
            tc.tile_pool(name="pwin", bufs=2, space="PSUM") as pwinpool,
            tc.tile_pool(name="pd", bufs=2, space="PSUM") as pdpool,
            tc.tile_pool(name="po", bufs=1, space="PSUM") as popool,
        ):
            ident = cpool.tile([128, 128], f16, tag="ident")
            w1a = cpool.tile([128, 128], f16, tag="w1a")
            w1b = cpool.tile([128, 128], f16, tag="w1b")
            w2a = cpool.tile([128, 128], f16, tag="w2a")
            w2b = cpool.tile([128, 128], f16, tag="w2b")
            b1t = cpool.tile([128, 1], f32, tag="b1")
            b2t = cpool.tile([128, 1], f32, tag="b2")
            wct = cpool.tile([128, NCOUT], f16, tag="wc")
            bct = cpool.tile([1, NCOUT], f16, tag="bc")
            ones1 = cpool.tile([1, ng], f16, tag="ones1")
            hg = cpool.tile([128, ng], f16, tag="hg")
            outs = cpool.tile([ng, NCOUT], f32, tag="outs")
            xt = bigpool.tile([128, ngg], f16, tag="xt")
            h1t = bigpool.tile([128, ngg], f16, tag="h1t")

            nc.sync.dma_start(out=ident[:], in_=id_in[:])
            nc.sync.dma_start(out=w1a[:], in_=w1a_in[:])
            nc.sync.dma_start(out=w1b[:], in_=w1b_in[:])
            nc.sync.dma_start(out=w2a[:], in_=w2a_in[:])
            nc.sync.dma_start(out=w2b[:], in_=w2b_in[:])
            nc.sync.dma_start(out=b1t[:], in_=b1_in[:])
            nc.sync.dma_start(out=b2t[:], in_=b2_in[:])
            nc.sync.dma_start(out=wct[:], in_=wc_in[:])
            nc.sync.dma_start(out=bct[:], in_=bc_in[:])
            nc.sync.dma_start(out=ones1[:], in_=ones_in[:])
            for s_ in range(ng):
                nc.sync.dma_start(
                    out=xt[:, s_ * gstride : (s_ + 1) * gstride],
                    in_=xt_in[:, s_ * gstride : (s_ + 1) * gstride],
                )

            for layer in range(2):
                srcT = xt if layer == 0 else h1t
                wa, wb = (w1a, w1b) if layer == 0 else (w2a, w2b)
                bt = b1t if layer == 0 else b2t

                for s in range(ng):
                    base = s * gstride

                    # node-major stationary chunks: stg[:, t, :] = srcT chunk^T
                    stg = stgpool.tile([128, nwin, 128], f16, tag="stg")
                    for t in range(nwin):
                        ptr = ptrpool.tile([128, 128], f32, tag="ptr")
                        nc.tensor.matmul(
                            ptr[:],
                            srcT[:, base + t * 128 : base + (t + 1) * 128],
                            ident[:],
                            start=True,
                            stop=True,
                        )
                        nc.vector.tensor_copy(stg[:, t, :], ptr[:])

                    # aggregation: Tx1T[:, slab] = sum_t stg_t^T @ S[t, slab]
                    tx1 = tx1pool.tile([128, gsize], f16, tag="tx1")
                    for q in range(nquad):
                        sb = sbpool.tile([128, nwin, nslab], f16, tag="sb")
                        r0 = ((s * nquad + q) * nwin) * 128
                        nc.sync.dma_start(
                            out=sb[:],
                            in_=sb_in[r0 : r0 + nwin * 128, :].rearrange(
                                "(p t) d -> p t d", t=nwin
                            ),
                        )
                        pwin = pwinpool.tile([128, nslab], f32, tag="pwin")
                        for t in range(nwin):
                            nc.tensor.matmul(
                                pwin[:],
                                stg[:, t, :],
                                sb[:, t, :],
                                start=(t == 0),
                                stop=(t == nwin - 1),
                            )
                        nc.vector.tensor_copy(
                            tx1[:, q * nslab : (q + 1) * nslab], pwin[:]
                        )

                    # dense: h = relu([Tx0, Tx1] @ W + b)
                    if layer == 1:
                        h2 = wpool.tile([128, gsize], f16, tag="h2")
                    for (noff, nlen) in ntiles:
                        pd = pdpool.tile([128, 512], f32, tag="pd")
                        nc.tensor.matmul(
                            pd[:, :nlen],
                            wa[:],
                            srcT[:, base + noff : base + noff + nlen],
                            start=True,
                            stop=False,
                        )
                        nc.tensor.matmul(
                            pd[:, :nlen],
                            wb[:],
                            tx1[:, noff : noff + nlen],
                            start=False,
                            stop=True,
                        )
                        dsttile = (
                            h1t[:, base + noff : base + noff + nlen]
                            if layer == 0
                            else h2[:, noff : noff + nlen]
                        )
                        nc.vector.tensor_scalar(
                            dsttile,
                            pd[:, :nlen],
                            bt[:],
                            0.0,
                            AL.add,
                            AL.max,
                        )
                    if layer == 0 and gstride > gsize:
                        nc.vector.memset(h1t[:, base + gsize : base + gstride], 0.0)
                    if layer == 1:
                        nc.vector.tensor_reduce(
                            hg[:, s : s + 1],
                            h2[:, :gsize],
                            mybir.AxisListType.X,
                            AL.max,
                        )

            # ---- readout: out = HG^T @ Wc + 1^T @ bc
            po = popool.tile([ng, NCOUT], f32, tag="po")
            nc.tensor.matmul(po[:], hg[:, :ng], wct[:], start=True, stop=False)
            nc.tensor.matmul(po[:], ones1[:], bct[:], start=False, stop=True)
            nc.vector.tensor_copy(outs[:], po[:])
            nc.sync.dma_start(out=out_dram[:], in_=outs[:])

    nc.compile()
    return nc


# ---------------------------------------------------------------- host glue
def _make_core_inputs(x, W1, b1, W2, b2, Wc, bc, pre, cfg):
    ng, gstride, gsize, nwin, nquad = (
        cfg["ng"],
        cfg["gstride"],
        cfg["gsize"],
        cfg["nwin"],
        cfg["nquad"],
    )
    nslab = cfg["nslab"]
    ngg = ng * gstride
    in_maps = []
    for c in range(NCORES):
        xt = np.zeros((128, ngg), dtype=np.float16)
        sblk = np.zeros((ng * nquad * nwin * 128, nslab), dtype=np.float16)
        sv = sblk.reshape(ng, nquad, 128, nwin, nslab)
        for s, g in enumerate(pre["slots"][c]):
            xg = x[g * gsize : (g + 1) * gsize]  # [gsize, 128]
            xt[:, s * gstride : s * gstride + gsize] = xg.T.astype(np.float16)
            # S_g [gstride, gsize] -> [q, p, t, d] (p-major rows: contiguous
            # 16-row reads per partition -> large DMA descriptors)
            Sg = pre["sblks"][g].reshape(nwin, 128, nquad, nslab)
            sv[s] = Sg.transpose(2, 1, 0, 3)
        in_maps.append(
            dict(
                XT=xt,
                SBLK=sblk,
                W1A=np.ascontiguousarray(W1[:128]).astype(np.float16),
                W1B=np.ascontiguousarray(W1[128:]).astype(np.float16),
                W2A=np.ascontiguousarray(W2[:128]).astype(np.float16),
                W2B=np.ascontiguousarray(W2[128:]).astype(np.float16),
                B1=b1.reshape(128, 1).astype(np.float32),
                B2=b2.reshape(128, 1).astype(np.float32),
                WC=Wc.astype(np.float16),
                BC=bc.reshape(1, NCOUT).astype(np.float16),
                ONES1=np.ones((1, ng), dtype=np.float16),
                IDENT=np.eye(128, dtype=np.float16),
            )
        )
    return in_maps


_CACHE = {}


def kernel(x, W1, b1, W2, b2, Wc, bc, src, dst, graph_ids, _trace=False):
    from concourse.bass_utils import run_bass_kernel_spmd

    x = np.asarray(x, dtype=np.float32)
    src = np.asarray(src).astype(np.int64)
    dst = np.asarray(dst).astype(np.int64)
    cfg = _default_cfg()

    pre = _preprocess(src, dst, cfg, N, B)
    key = "prog"
    if key not in _CACHE:
        _CACHE[key] = _build_program(cfg)
    nc = _CACHE[key]

    in_maps = _make_core_inputs(
        x,
        np.asarray(W1, np.float32),
        np.asarray(b1, np.float32),
        np.asarray(W2, np.float32),
        np.asarray(b2, np.float32),
        np.asarray(Wc, np.float32),
        np.asarray(bc, np.float32),
        pre,
        cfg,
    )
    res = run_bass_kernel_spmd(nc, in_maps, list(range(NCORES)), trace=_trace)

    out = np.zeros((B, NCOUT), dtype=np.float32)
    for c in range(NCORES):
        oc = res.results[c]["OUT"]
        for s, g in enumerate(pre["slots"][c]):
            out[g] = oc[s]
    if _trace:
        kernel._last_exec_ns = res.exec_time_ns
    return out



# revision 2
# speedup vs baseline: 1.8022x; 1.8022x over previous
"""ChebNet (K=2) graph classifier on 8 Trainium2 NeuronCores.

Strategy (graph/data parallel, zero halo):
  - The 50 batched graphs are independent, so graphs are assigned whole to
    cores (7-graph slots; 50 = 2*7 + 6*6).  One SPMD program runs on all 8
    cores; cores with fewer graphs chew zero blocks.
  - The normalized aggregation Tx1 = -D^-1/2 A D^-1/2 feat is a dense
    per-graph matmul against the edge-count matrix C (structural, built
    host-side).  C is stored as EXACT fp8e4 small-int counts and streamed
    from HBM ONCE per graph, resident in SBUF across both Chebyshev layers
    (the baseline streamed scaled fp16 blocks twice: 4x the HBM traffic).
  - The degree scalings are factored out of C:  agg = C^T (dinv*feat),
    Tx1 = -dinv[dst] * agg.  The src scale rides the node-major stationary
    tiles (host-prescaled fp8 for layer 1, a fused DVE tensor_scalar after
    the on-chip transposes for layer 2); the dst scale is a host-staged
    -dinv broadcast tile multiplied into the PSUM->SBUF copy
    (scalar_tensor_tensor), replacing the plain copy at zero extra cost.
  - With both aggregation operands in fp8, the matmuls run in DoubleRow
    perf mode (256-deep contraction per pass) at free-dim 512/464, ~1.4-2x
    the fp16 PE rate.  Dense Chebyshev layers, bias+relu, max-pool readout
    and the classifier stay fp16/fp32 on-device.
"""

import sys

if "/opt/trn_rl_repo" not in sys.path:
    sys.path.insert(0, "/opt/trn_rl_repo")

import numpy as np
import ml_dtypes

# ---------------------------------------------------------------- constants
N = 100_000
E = 1_600_000
B = 50
GSIZE = 2000
D = 128  # IN == HID == 128
NCOUT = 10
NCORES = 8
NG = 7  # graph slots per core (50 = 2*7 + 6*6)
NWIN = 16  # src windows of 128
GSTRIDE = NWIN * 128  # 2048
QUADS = [(0, 512), (512, 512), (1024, 512), (1536, 464)]  # dst tiling of 2000
SROW = NWIN * GSIZE  # S cols per slot (quad-major: [q][t][qn])

F8 = ml_dtypes.float8_e4m3


# ---------------------------------------------------------------- host prep
def _preprocess(src, dst):
    """Structural preprocessing: graph->core assignment, degrees, and
    per-graph edge-count blocks [128, 16, 2000] (partition-major windows)."""
    deg = np.bincount(dst, minlength=N)
    dinv = (np.clip(deg.astype(np.float64), 1.0, None) ** -0.5).astype(np.float32)

    order = [0, 2, 1, 3, 4, 5, 6, 7]  # extra graphs land on cores 0 and 2
    slots = [[] for _ in range(NCORES)]
    for g in range(B):
        slots[order[g % NCORES]].append(g)

    g_of_e = dst // GSIZE
    flat = (src - g_of_e * GSIZE) * np.int64(GSIZE) + (dst - g_of_e * GSIZE)
    cblks = []
    for g in range(B):
        m = g_of_e == g
        cnt = np.bincount(flat[m], minlength=GSTRIDE * GSIZE).astype(np.float32)
        # [2048 src, 2000 dst] -> [128 p, 16 t, 2000 d]
        c = cnt.reshape(NWIN, 128, GSIZE).transpose(1, 0, 2)
        cblks.append(c.astype(F8))
    return dict(slots=slots, cblks=cblks, dinv=dinv)


# ---------------------------------------------------------------- program
def _build_program():
    from concourse import bacc, mybir, tile

    f8 = mybir.dt.float8e4
    f16 = mybir.dt.float16
    f32 = mybir.dt.float32
    AL = mybir.AluOpType
    DR = mybir.MatmulPerfMode.DoubleRow

    nc = bacc.Bacc(None, target_bir_lowering=False)

    xg_in = nc.declare_dram_parameter("XG", [128, NG * GSIZE], f16, isOutput=False)
    ynm_in = nc.declare_dram_parameter("YNM8", [128, NG * GSTRIDE], f8, isOutput=False)
    sc_in = nc.declare_dram_parameter("SC8", [128, NG * SROW], f8, isOutput=False)
    ndb_in = nc.declare_dram_parameter("NDB", [128, NG * GSIZE], f16, isOutput=False)
    dsrc_in = nc.declare_dram_parameter("DSRC", [128, NG * NWIN], f32, isOutput=False)
    w1a_in = nc.declare_dram_parameter("W1A", [128, 128], f16, isOutput=False)
    w1b_in = nc.declare_dram_parameter("W1B", [128, 128], f16, isOutput=False)
    w2a_in = nc.declare_dram_parameter("W2A", [128, 128], f16, isOutput=False)
    w2b_in = nc.declare_dram_parameter("W2B", [128, 128], f16, isOutput=False)
    b1_in = nc.declare_dram_parameter("B1", [128, 1], f32, isOutput=False)
    b2_in = nc.declare_dram_parameter("B2", [128, 1], f32, isOutput=False)
    wc_in = nc.declare_dram_parameter("WC", [128, NCOUT], f16, isOutput=False)
    bc_in = nc.declare_dram_parameter("BC", [1, NCOUT], f16, isOutput=False)
    ones_in = nc.declare_dram_parameter("ONES1", [1, NG], f16, isOutput=False)
    id_in = nc.declare_dram_parameter("IDENT", [128, 128], f16, isOutput=False)
    out_dram = nc.declare_dram_parameter("OUT", [NG, NCOUT], f32, isOutput=True)

    with tile.TileContext(nc) as tc:
        with (
            tc.tile_pool(name="const", bufs=1) as cpool,
            tc.tile_pool(name="sblk", bufs=2) as sbpool,
            tc.tile_pool(name="stg", bufs=2) as stgpool,
            tc.tile_pool(name="tx1", bufs=2) as tx1pool,
            tc.tile_pool(name="h1", bufs=2) as h1pool,
            tc.tile_pool(name="h2", bufs=2) as h2pool,
            tc.tile_pool(name="ptr", bufs=2, space="PSUM") as ptrpool,
            tc.tile_pool(name="pwin", bufs=2, space="PSUM") as pwinpool,
            tc.tile_pool(name="pd", bufs=2, space="PSUM") as pdpool,
            tc.tile_pool(name="po", bufs=1, space="PSUM") as popool,
        ):
            ident = cpool.tile([128, 128], f16, tag="ident")
            w1a = cpool.tile([128, 128], f16, tag="w1a")
            w1b = cpool.tile([128, 128], f16, tag="w1b")
            w2a = cpool.tile([128, 128], f16, tag="w2a")
            w2b = cpool.tile([128, 128], f16, tag="w2b")
            b1t = cpool.tile([128, 1], f32, tag="b1")
            b2t = cpool.tile([128, 1], f32, tag="b2")
            wct = cpool.tile([128, NCOUT], f16, tag="wc")
            bct = cpool.tile([1, NCOUT], f16, tag="bc")
            ones1 = cpool.tile([1, NG], f16, tag="ones1")
            hg = cpool.tile([128, NG], f16, tag="hg")
            outs = cpool.tile([NG, NCOUT], f32, tag="outs")
            dsrc = cpool.tile([128, NG * NWIN], f32, tag="dsrc")
            xgall = cpool.tile([128, NG * GSIZE], f16, tag="xgall")
            ynm = cpool.tile([128, NG * NWIN, 128], f8, tag="ynm")
            ndb = cpool.tile([128, NG * GSIZE], f16, tag="ndb")

            nc.sync.dma_start(out=ident[:], in_=id_in[:])
            nc.sync.dma_start(out=w1a[:], in_=w1a_in[:])
            nc.sync.dma_start(out=w1b[:], in_=w1b_in[:])
            nc.sync.dma_start(out=w2a[:], in_=w2a_in[:])
            nc.sync.dma_start(out=w2b[:], in_=w2b_in[:])
            nc.sync.dma_start(out=b1t[:], in_=b1_in[:])
            nc.sync.dma_start(out=b2t[:], in_=b2_in[:])
            nc.sync.dma_start(out=wct[:], in_=wc_in[:])
            nc.sync.dma_start(out=bct[:], in_=bc_in[:])
            nc.sync.dma_start(out=ones1[:], in_=ones_in[:])
            nc.sync.dma_start(out=dsrc[:], in_=dsrc_in[:])
            for s in range(NG):
                gsl = slice(s * GSIZE, (s + 1) * GSIZE)
                nc.sync.dma_start(out=xgall[:, gsl], in_=xg_in[:, gsl])
                nc.sync.dma_start(out=ndb[:, gsl], in_=ndb_in[:, gsl])
                nc.sync.dma_start(
                    out=ynm[:, s * NWIN : (s + 1) * NWIN, :],
                    in_=ynm_in[:, s * GSTRIDE : (s + 1) * GSTRIDE].rearrange(
                        "p (t f) -> p t f", f=128
                    ),
                )

            for s in range(NG):
                # S blocks for this graph: one tile per dst quad, resident
                # across both layers.  Host layout is quad-major so each
                # partition reads one contiguous 16*qn run per quad.
                sbq = []
                for qi, (qoff, qn) in enumerate(QUADS):
                    sb = sbpool.tile([128, NWIN, qn], f8, tag=f"sb{qi}")
                    c0 = s * SROW + qoff * NWIN
                    nc.sync.dma_start(
                        out=sb[:],
                        in_=sc_in[:, c0 : c0 + NWIN * qn].rearrange(
                            "p (t d) -> p t d", t=NWIN
                        ),
                    )
                    sbq.append(sb)

                h1 = h1pool.tile([128, GSTRIDE], f16, tag="h1")
                h2 = h2pool.tile([128, GSIZE], f16, tag="h2")

                for layer in range(2):
                    if layer == 0:
                        stg3, tbase = ynm, s * NWIN
                    else:
                        # node-major dinv-scaled fp8 copy of h1 via PE
                        # transposes + fused DVE scale/cast
                        stg2 = stgpool.tile([128, NWIN, 128], f8, tag="stg2")
                        for t in range(NWIN):
                            ptr = ptrpool.tile([128, 128], f32, tag="ptr")
                            nc.tensor.matmul(
                                ptr[:],
                                h1[:, t * 128 : (t + 1) * 128],
                                ident[:],
                                start=True,
                                stop=True,
                            )
                            nc.vector.tensor_scalar(
                                stg2[:, t, :],
                                ptr[:],
                                dsrc[:, s * NWIN + t : s * NWIN + t + 1],
                                None,
                                AL.mult,
                            )
                        stg3, tbase = stg2, 0

                    # aggregation: tx1[f, d] = -dinv[d] * sum_s y[s, f] C[s, d]
                    tx1 = tx1pool.tile([128, GSIZE], f16, tag="tx1")
                    for qi, (qoff, qn) in enumerate(QUADS):
                        pwin = pwinpool.tile([128, 512], f32, tag="pwin")
                        for th in range(NWIN // 2):
                            nc.tensor.matmul(
                                pwin[:, :qn],
                                stg3[:, tbase + 2 * th : tbase + 2 * th + 2, :],
                                sbq[qi][:, 2 * th : 2 * th + 2, :],
                                start=(th == 0),
                                stop=(th == NWIN // 2 - 1),
                                perf_mode=DR,
                            )
                        nc.vector.scalar_tensor_tensor(
                            tx1[:, qoff : qoff + qn],
                            pwin[:, :qn],
                            1.0,
                            ndb[:, s * GSIZE + qoff : s * GSIZE + qoff + qn],
                            AL.mult,
                            AL.mult,
                        )

                    # dense: h = relu([Tx0, Tx1] @ W + b)
                    wa, wb = (w1a, w1b) if layer == 0 else (w2a, w2b)
                    bt = b1t if layer == 0 else b2t
                    for qoff, qn in QUADS:
                        pd = pdpool.tile([128, 512], f32, tag="pd")
                        rhs0 = (
                            xgall[:, s * GSIZE + qoff : s * GSIZE + qoff + qn]
                            if layer == 0
                            else h1[:, qoff : qoff + qn]
                        )
                        nc.tensor.matmul(
                            pd[:, :qn], wa[:], rhs0, start=True, stop=False
                        )
                        nc.tensor.matmul(
                            pd[:, :qn],
                            wb[:],
                            tx1[:, qoff : qoff + qn],
                            start=False,
                            stop=True,
                        )
                        dst_ap = (
                            h1[:, qoff : qoff + qn]
                            if layer == 0
                            else h2[:, qoff : qoff + qn]
                        )
                        nc.vector.tensor_scalar(
                            dst_ap, pd[:, :qn], bt[:], 0.0, AL.add, AL.max
                        )
                    if layer == 0:
                        nc.vector.memset(h1[:, GSIZE:GSTRIDE], 0.0)
                    else:
                        nc.vector.tensor_reduce(
                            hg[:, s : s + 1],
                            h2[:, :GSIZE],
                            mybir.AxisListType.X,
                            AL.max,
                        )

            # ---- readout: out = HG^T @ Wc + 1^T @ bc
            po = popool.tile([NG, NCOUT], f32, tag="po")
            nc.tensor.matmul(po[:], hg[:, :NG], wct[:], start=True, stop=False)
            nc.tensor.matmul(po[:], ones1[:], bct[:], start=False, stop=True)
            nc.vector.tensor_copy(outs[:], po[:])
            nc.sync.dma_start(out=out_dram[:], in_=outs[:])

    nc.compile()
    return nc


# ---------------------------------------------------------------- host glue
def _make_core_inputs(x, W1, b1, W2, b2, Wc, bc, pre):
    dinv = pre["dinv"]
    in_maps = []
    for c in range(NCORES):
        xg = np.zeros((128, NG * GSIZE), dtype=np.float16)
        ynm = np.zeros((128, NG * GSTRIDE), dtype=F8)
        sc = np.zeros((128, NG * SROW), dtype=F8)
        ndb = np.zeros((128, NG * GSIZE), dtype=np.float16)
        dsrc = np.zeros((128, NG * NWIN), dtype=np.float32)
        for s, g in enumerate(pre["slots"][c]):
            xgf = x[g * GSIZE : (g + 1) * GSIZE]  # [2000, 128] f32
            dv = dinv[g * GSIZE : (g + 1) * GSIZE]  # [2000]
            xg[:, s * GSIZE : (s + 1) * GSIZE] = xgf.T.astype(np.float16)
            ndb[:, s * GSIZE : (s + 1) * GSIZE] = np.broadcast_to(
                (-dv).astype(np.float16), (128, GSIZE)
            )
            y = np.zeros((GSTRIDE, 128), dtype=np.float32)
            y[:GSIZE] = dv[:, None] * xgf
            # [2048, 128] -> [128 p, 16 t, 128 f]
            ynm[:, s * GSTRIDE : (s + 1) * GSTRIDE] = (
                y.reshape(NWIN, 128, 128).transpose(1, 0, 2).reshape(128, GSTRIDE)
            ).astype(F8)
            dvp = np.zeros(GSTRIDE, dtype=np.float32)
            dvp[:GSIZE] = dv
            dsrc[:, s * NWIN : (s + 1) * NWIN] = dvp.reshape(NWIN, 128).T
            cb = pre["cblks"][g]  # [128, 16, 2000] f8
            parts = [
                cb[:, :, qoff : qoff + qn].reshape(128, NWIN * qn)
                for qoff, qn in QUADS
            ]
            sc[:, s * SROW : (s + 1) * SROW] = np.concatenate(parts, axis=1)
        in_maps.append(
            dict(
                XG=xg,
                YNM8=ynm,
                SC8=sc,
                NDB=ndb,
                DSRC=dsrc,
                W1A=np.ascontiguousarray(W1[:128]).astype(np.float16),
                W1B=np.ascontiguousarray(W1[128:]).astype(np.float16),
                W2A=np.ascontiguousarray(W2[:128]).astype(np.float16),
                W2B=np.ascontiguousarray(W2[128:]).astype(np.float16),
                B1=b1.reshape(128, 1).astype(np.float32),
                B2=b2.reshape(128, 1).astype(np.float32),
                WC=Wc.astype(np.float16),
                BC=bc.reshape(1, NCOUT).astype(np.float16),
                ONES1=np.ones((1, NG), dtype=np.float16),
                IDENT=np.eye(128, dtype=np.float16),
            )
        )
    return in_maps


_CACHE = {}


def kernel(x, W1, b1, W2, b2, Wc, bc, src, dst, graph_ids, _trace=False):
    from concourse.bass_utils import run_bass_kernel_spmd

    x = np.asarray(x, dtype=np.float32)
    src = np.asarray(src).astype(np.int64)
    dst = np.asarray(dst).astype(np.int64)

    pre = _preprocess(src, dst)
    if "prog" not in _CACHE:
        _CACHE["prog"] = _build_program()
    nc = _CACHE["prog"]

    in_maps = _make_core_inputs(
        x,
        np.asarray(W1, np.float32),
        np.asarray(b1, np.float32),
        np.asarray(W2, np.float32),
        np.asarray(b2, np.float32),
        np.asarray(Wc, np.float32),
        np.asarray(bc, np.float32),
        pre,
    )
    res = run_bass_kernel_spmd(nc, in_maps, list(range(NCORES)), trace=_trace)

    out = np.zeros((B, NCOUT), dtype=np.float32)
    for c in range(NCORES):
        oc = res.results[c]["OUT"]
        for s, g in enumerate(pre["slots"][c]):
            out[g] = oc[s]
    if _trace:
        kernel._last_exec_ns = res.exec_time_ns
    return out


# revision 9
# speedup vs baseline: 2.0082x; 1.1143x over previous
"""ChebNet (K=2) graph classifier on 8 Trainium2 NeuronCores.

Strategy (graph/data parallel, zero halo):
  - The 50 batched graphs are independent, so graphs are assigned whole to
    cores (7-graph slots; 50 = 2*7 + 6*6).  One SPMD program runs on all 8
    cores; cores with fewer graphs chew zero blocks.
  - The normalized aggregation Tx1 = -D^-1/2 A D^-1/2 feat is a dense
    per-graph matmul against the edge-count matrix C (structural, built
    host-side).  C is stored as EXACT fp8e4 small-int counts and streamed
    from HBM ONCE per graph, resident in SBUF across both Chebyshev layers
    (the baseline streamed scaled fp16 blocks twice: 4x the HBM traffic).
  - The degree scalings are factored out of C:  agg = C^T (dinv*feat),
    Tx1 = -dinv[dst] * agg.  The src scale rides the node-major stationary
    tiles (host-prescaled fp8 for layer 1, a fused DVE tensor_scalar after
    the on-chip transposes for layer 2); the dst scale is a host-staged
    -dinv broadcast tile multiplied into the PSUM->SBUF copy
    (scalar_tensor_tensor), replacing the plain copy at zero extra cost.
  - With both aggregation operands in fp8, the matmuls run in DoubleRow
    perf mode (256-deep contraction per pass) at free-dim 512/464, ~1.4-2x
    the fp16 PE rate.  Dense Chebyshev layers, bias+relu, max-pool readout
    and the classifier stay fp16/fp32 on-device.
"""

import sys

if "/opt/trn_rl_repo" not in sys.path:
    sys.path.insert(0, "/opt/trn_rl_repo")

import numpy as np
import ml_dtypes

# ---------------------------------------------------------------- constants
N = 100_000
E = 1_600_000
B = 50
GSIZE = 2000
D = 128  # IN == HID == 128
NCOUT = 10
NCORES = 8
NG = 7  # graph slots per core (50 = 2*7 + 6*6)
NWIN = 16  # src windows of 128
GSTRIDE = NWIN * 128  # 2048
QUADS = [(0, 512), (512, 512), (1024, 512), (1536, 464)]  # dst tiling of 2000
SROW = NWIN * GSIZE  # S cols per slot (quad-major: [q][t][qn])

F8 = ml_dtypes.float8_e4m3


# ---------------------------------------------------------------- host prep
def _preprocess(src, dst):
    """Structural preprocessing: graph->core assignment, degrees, and
    per-graph edge-count blocks [128, 16, 2000] (partition-major windows)."""
    deg = np.bincount(dst, minlength=N)
    dinv = (np.clip(deg.astype(np.float64), 1.0, None) ** -0.5).astype(np.float32)

    order = [0, 2, 1, 3, 4, 5, 6, 7]  # extra graphs land on cores 0 and 2
    slots = [[] for _ in range(NCORES)]
    for g in range(B):
        slots[order[g % NCORES]].append(g)

    g_of_e = dst // GSIZE
    flat = (src - g_of_e * GSIZE) * np.int64(GSIZE) + (dst - g_of_e * GSIZE)
    cblks = []
    for g in range(B):
        m = g_of_e == g
        cnt = np.bincount(flat[m], minlength=GSTRIDE * GSIZE).astype(np.float32)
        # [2048 src, 2000 dst] -> [128 p, 16 t, 2000 d]
        c = cnt.reshape(NWIN, 128, GSIZE).transpose(1, 0, 2)
        cblks.append(c.astype(F8))
    return dict(slots=slots, cblks=cblks, dinv=dinv)


# ---------------------------------------------------------------- program
def _build_program():
    from concourse import bacc, mybir, tile

    f8 = mybir.dt.float8e4
    f16 = mybir.dt.float16
    f32 = mybir.dt.float32
    AL = mybir.AluOpType
    DR = mybir.MatmulPerfMode.DoubleRow

    nc = bacc.Bacc(None, target_bir_lowering=False)

    xg_in = nc.declare_dram_parameter("XG", [128, NG * GSIZE], f16, isOutput=False)
    ynm_in = nc.declare_dram_parameter("YNM8", [128, NG * GSTRIDE], f8, isOutput=False)
    sc_in = nc.declare_dram_parameter("SC8", [128, NG * SROW], f8, isOutput=False)
    ndb_in = nc.declare_dram_parameter("NDB", [128, NG * GSIZE], f16, isOutput=False)
    dsb_in = nc.declare_dram_parameter(
        "DSRCB", [128, NG * GSTRIDE], f16, isOutput=False
    )
    w1a_in = nc.declare_dram_parameter("W1A", [128, 128], f16, isOutput=False)
    w1b_in = nc.declare_dram_parameter("W1B", [128, 128], f16, isOutput=False)
    w2a_in = nc.declare_dram_parameter("W2A", [128, 128], f16, isOutput=False)
    w2b_in = nc.declare_dram_parameter("W2B", [128, 128], f16, isOutput=False)
    b1_in = nc.declare_dram_parameter("B1", [128, 1], f32, isOutput=False)
    b2_in = nc.declare_dram_parameter("B2", [128, 1], f32, isOutput=False)
    wc_in = nc.declare_dram_parameter("WC", [128, NCOUT], f16, isOutput=False)
    bc_in = nc.declare_dram_parameter("BC", [1, NCOUT], f16, isOutput=False)
    ones_in = nc.declare_dram_parameter("ONES1", [1, NG], f16, isOutput=False)
    id_in = nc.declare_dram_parameter("IDENT", [128, 128], f16, isOutput=False)
    out_dram = nc.declare_dram_parameter("OUT", [NG, NCOUT], f32, isOutput=True)

    with tile.TileContext(nc) as tc:
        with (
            tc.tile_pool(name="const", bufs=1) as cpool,
            tc.tile_pool(name="sblk", bufs=2) as sbpool,
            tc.tile_pool(name="gin", bufs=2) as ginpool,
            tc.tile_pool(name="stg", bufs=2) as stgpool,
            tc.tile_pool(name="tx1", bufs=2) as tx1pool,
            tc.tile_pool(name="h1", bufs=2) as h1pool,
            tc.tile_pool(name="h2", bufs=2) as h2pool,
            tc.tile_pool(name="ptr", bufs=2, space="PSUM") as ptrpool,
            tc.tile_pool(name="pwin", bufs=4, space="PSUM") as pwinpool,
            tc.tile_pool(name="pd", bufs=2, space="PSUM") as pdpool,
        ):
            ident = cpool.tile([128, 128], f16, tag="ident")
            w1a = cpool.tile([128, 128], f16, tag="w1a")
            w1b = cpool.tile([128, 128], f16, tag="w1b")
            w2a = cpool.tile([128, 128], f16, tag="w2a")
            w2b = cpool.tile([128, 128], f16, tag="w2b")
            b1t = cpool.tile([128, 1], f32, tag="b1")
            b2t = cpool.tile([128, 1], f32, tag="b2")
            wct = cpool.tile([128, NCOUT], f16, tag="wc")
            bct = cpool.tile([1, NCOUT], f16, tag="bc")
            ones1 = cpool.tile([1, NG], f16, tag="ones1")
            hg = cpool.tile([128, NG], f16, tag="hg")
            outs = cpool.tile([NG, NCOUT], f32, tag="outs")
            dsrcb = cpool.tile([128, NG * GSTRIDE], f16, tag="dsrcb")

            nc.sync.dma_start(out=ident[:], in_=id_in[:])
            nc.sync.dma_start(out=w1a[:], in_=w1a_in[:])
            nc.sync.dma_start(out=w1b[:], in_=w1b_in[:])
            nc.sync.dma_start(out=w2a[:], in_=w2a_in[:])
            nc.sync.dma_start(out=w2b[:], in_=w2b_in[:])
            nc.sync.dma_start(out=b1t[:], in_=b1_in[:])
            nc.sync.dma_start(out=b2t[:], in_=b2_in[:])
            nc.sync.dma_start(out=wct[:], in_=wc_in[:])
            nc.sync.dma_start(out=bct[:], in_=bc_in[:])
            nc.sync.dma_start(out=ones1[:], in_=ones_in[:])
            for s in range(NG):
                nc.sync.dma_start(
                    out=dsrcb[:, s * GSTRIDE : (s + 1) * GSTRIDE],
                    in_=dsb_in[:, s * GSTRIDE : (s + 1) * GSTRIDE],
                )

            for s in range(NG):
                # Per-graph inputs, loaded just-in-time (double-buffered so
                # graph s+1 streams while s computes; avoids a startup stall
                # on one huge const load).
                xg = ginpool.tile([128, GSIZE], f16, tag="xg")
                ndb = ginpool.tile([128, GSIZE], f16, tag="ndb")
                ynm = ginpool.tile([128, NWIN, 128], f8, tag="ynm")
                nc.sync.dma_start(
                    out=xg[:], in_=xg_in[:, s * GSIZE : (s + 1) * GSIZE]
                )
                nc.sync.dma_start(
                    out=ndb[:], in_=ndb_in[:, s * GSIZE : (s + 1) * GSIZE]
                )
                nc.sync.dma_start(
                    out=ynm[:],
                    in_=ynm_in[:, s * GSTRIDE : (s + 1) * GSTRIDE].rearrange(
                        "p (t f) -> p t f", f=128
                    ),
                )
                # S blocks for this graph: one tile per dst quad, resident
                # across both layers.  Host layout is quad-major so each
                # partition reads contiguous runs; each quad is split into
                # two DMAs to spread across DMA engines.
                sbq = []
                for qi, (qoff, qn) in enumerate(QUADS):
                    sb = sbpool.tile([128, NWIN, qn], f8, tag=f"sb{qi}")
                    c0 = s * SROW + qoff * NWIN
                    half = (NWIN // 2) * qn
                    for hh in range(2):
                        nc.sync.dma_start(
                            out=sb[:, hh * (NWIN // 2) : (hh + 1) * (NWIN // 2), :],
                            in_=sc_in[
                                :, c0 + hh * half : c0 + (hh + 1) * half
                            ].rearrange("p (t d) -> p t d", t=NWIN // 2),
                        )
                    sbq.append(sb)

                h1 = h1pool.tile([128, GSTRIDE], f16, tag="h1")
                h2 = h2pool.tile([128, GSIZE], f16, tag="h2")

                for layer in range(2):
                    if layer == 0:
                        stg3 = ynm
                    else:
                        # node-major dinv-scaled fp8 copy of h1: PE
                        # transposes batched 4-per-PSUM-bank (one matmul
                        # group, disjoint column ranges), then one fused
                        # DVE scale/cast per batch
                        stg2 = stgpool.tile([128, NWIN, 128], f8, tag="stg2")
                        for j in range(NWIN // 4):
                            ptr4 = ptrpool.tile([128, 512], f32, tag="ptr")
                            for k in range(4):
                                t = j * 4 + k
                                nc.tensor.matmul(
                                    ptr4[:, k * 128 : (k + 1) * 128],
                                    h1[:, t * 128 : (t + 1) * 128],
                                    ident[:],
                                    start=(k == 0),
                                    stop=(k == 3),
                                    skip_group_check=True,
                                )
                            nc.vector.scalar_tensor_tensor(
                                stg2[:, j * 4 : (j + 1) * 4, :],
                                ptr4[:],
                                1.0,
                                dsrcb[
                                    :, s * GSTRIDE + j * 512 : s * GSTRIDE + (j + 1) * 512
                                ],
                                AL.mult,
                                AL.mult,
                            )
                        stg3 = stg2

                    # aggregation: tx1[f, d] = -dinv[d] * sum_s y[s, f] C[s, d]
                    # th-outer so each DoubleRow stationary is reused across
                    # the 4 dst-quad accumulators (amortizes LDWEIGHTS)
                    tx1 = tx1pool.tile([128, GSIZE], f16, tag="tx1")
                    pwins = [
                        pwinpool.tile([128, 512], f32, tag="pwin", name=f"pwin{qi}")
                        for qi in range(len(QUADS))
                    ]
                    for th in range(NWIN // 2):
                        for qi, (qoff, qn) in enumerate(QUADS):
                            nc.tensor.matmul(
                                pwins[qi][:, :qn],
                                stg3[:, 2 * th : 2 * th + 2, :],
                                sbq[qi][:, 2 * th : 2 * th + 2, :],
                                start=(th == 0),
                                stop=(th == NWIN // 2 - 1),
                                perf_mode=DR,
                                skip_group_check=True,
                            )
                    for qi, (qoff, qn) in enumerate(QUADS):
                        nc.vector.scalar_tensor_tensor(
                            tx1[:, qoff : qoff + qn],
                            pwins[qi][:, :qn],
                            1.0,
                            ndb[:, qoff : qoff + qn],
                            AL.mult,
                            AL.mult,
                        )

                    # dense: h = relu([Tx0, Tx1] @ W + b); bias+relu on the
                    # (otherwise idle) scalar engine
                    wa, wb = (w1a, w1b) if layer == 0 else (w2a, w2b)
                    bt = b1t if layer == 0 else b2t
                    for qoff, qn in QUADS:
                        pd = pdpool.tile([128, 512], f32, tag="pd")
                        rhs0 = (
                            xg[:, qoff : qoff + qn]
                            if layer == 0
                            else h1[:, qoff : qoff + qn]
                        )
                        nc.tensor.matmul(
                            pd[:, :qn], wa[:], rhs0, start=True, stop=False
                        )
                        nc.tensor.matmul(
                            pd[:, :qn],
                            wb[:],
                            tx1[:, qoff : qoff + qn],
                            start=False,
                            stop=True,
                        )
                        dst_ap = (
                            h1[:, qoff : qoff + qn]
                            if layer == 0
                            else h2[:, qoff : qoff + qn]
                        )
                        nc.scalar.activation(
                            dst_ap,
                            pd[:, :qn],
                            mybir.ActivationFunctionType.Relu,
                            bias=bt[:],
                            scale=1.0,
                        )
                    if layer == 0:
                        nc.vector.memset(h1[:, GSIZE:GSTRIDE], 0.0)
                    else:
                        nc.vector.tensor_reduce(
                            hg[:, s : s + 1],
                            h2[:, :GSIZE],
                            mybir.AxisListType.X,
                            AL.max,
                        )

            # ---- readout: out = HG^T @ Wc + 1^T @ bc
            pot = pdpool.tile([128, 512], f32, tag="pd")
            po = pot[:NG, :NCOUT]
            nc.tensor.matmul(po, hg[:, :NG], wct[:], start=True, stop=False)
            nc.tensor.matmul(po, ones1[:], bct[:], start=False, stop=True)
            nc.vector.tensor_copy(outs[:], po)
            nc.sync.dma_start(out=out_dram[:], in_=outs[:])

    nc.compile()
    return nc


# ---------------------------------------------------------------- host glue
def _make_core_inputs(x, W1, b1, W2, b2, Wc, bc, pre):
    dinv = pre["dinv"]
    in_maps = []
    for c in range(NCORES):
        xg = np.zeros((128, NG * GSIZE), dtype=np.float16)
        ynm = np.zeros((128, NG * GSTRIDE), dtype=F8)
        sc = np.zeros((128, NG * SROW), dtype=F8)
        ndb = np.zeros((128, NG * GSIZE), dtype=np.float16)
        dsrcb = np.zeros((128, NG * GSTRIDE), dtype=np.float16)
        for s, g in enumerate(pre["slots"][c]):
            xgf = x[g * GSIZE : (g + 1) * GSIZE]  # [2000, 128] f32
            dv = dinv[g * GSIZE : (g + 1) * GSIZE]  # [2000]
            xg[:, s * GSIZE : (s + 1) * GSIZE] = xgf.T.astype(np.float16)
            ndb[:, s * GSIZE : (s + 1) * GSIZE] = np.broadcast_to(
                (-dv).astype(np.float16), (128, GSIZE)
            )
            y = np.zeros((GSTRIDE, 128), dtype=np.float32)
            y[:GSIZE] = dv[:, None] * xgf
            # [2048, 128] -> [128 p, 16 t, 128 f]
            ynm[:, s * GSTRIDE : (s + 1) * GSTRIDE] = (
                y.reshape(NWIN, 128, 128).transpose(1, 0, 2).reshape(128, GSTRIDE)
            ).astype(F8)
            dvp = np.zeros(GSTRIDE, dtype=np.float32)
            dvp[:GSIZE] = dv
            # dsrcb[p, t*128 + f] = dinv[t*128 + p] (node-major, bcast over f)
            dsrcb[:, s * GSTRIDE : (s + 1) * GSTRIDE] = np.broadcast_to(
                dvp.reshape(NWIN, 128).T.astype(np.float16)[:, :, None],
                (128, NWIN, 128),
            ).reshape(128, GSTRIDE)
            cb = pre["cblks"][g]  # [128, 16, 2000] f8
            parts = [
                cb[:, :, qoff : qoff + qn].reshape(128, NWIN * qn)
                for qoff, qn in QUADS
            ]
            sc[:, s * SROW : (s + 1) * SROW] = np.concatenate(parts, axis=1)
        in_maps.append(
            dict(
                XG=xg,
                YNM8=ynm,
                SC8=sc,
                NDB=ndb,
                DSRCB=dsrcb,
                W1A=np.ascontiguousarray(W1[:128]).astype(np.float16),
                W1B=np.ascontiguousarray(W1[128:]).astype(np.float16),
                W2A=np.ascontiguousarray(W2[:128]).astype(np.float16),
                W2B=np.ascontiguousarray(W2[128:]).astype(np.float16),
                B1=b1.reshape(128, 1).astype(np.float32),
                B2=b2.reshape(128, 1).astype(np.float32),
                WC=Wc.astype(np.float16),
                BC=bc.reshape(1, NCOUT).astype(np.float16),
                ONES1=np.ones((1, NG), dtype=np.float16),
                IDENT=np.eye(128, dtype=np.float16),
            )
        )
    return in_maps


_CACHE = {}


def kernel(x, W1, b1, W2, b2, Wc, bc, src, dst, graph_ids, _trace=False):
    from concourse.bass_utils import run_bass_kernel_spmd

    x = np.asarray(x, dtype=np.float32)
    src = np.asarray(src).astype(np.int64)
    dst = np.asarray(dst).astype(np.int64)

    pre = _preprocess(src, dst)
    if "prog" not in _CACHE:
        _CACHE["prog"] = _build_program()
    nc = _CACHE["prog"]

    in_maps = _make_core_inputs(
        x,
        np.asarray(W1, np.float32),
        np.asarray(b1, np.float32),
        np.asarray(W2, np.float32),
        np.asarray(b2, np.float32),
        np.asarray(Wc, np.float32),
        np.asarray(bc, np.float32),
        pre,
    )
    res = run_bass_kernel_spmd(nc, in_maps, list(range(NCORES)), trace=_trace)

    out = np.zeros((B, NCOUT), dtype=np.float32)
    for c in range(NCORES):
        oc = res.results[c]["OUT"]
        for s, g in enumerate(pre["slots"][c]):
            out[g] = oc[s]
    if _trace:
        kernel._last_exec_ns = res.exec_time_ns
    return out


# revision 12
# speedup vs baseline: 2.0810x; 1.0363x over previous
"""ChebNet (K=2) graph classifier on 8 Trainium2 NeuronCores.

Strategy (graph/data parallel, zero halo):
  - The 50 batched graphs are independent, so graphs are assigned whole to
    cores (7-graph slots; 50 = 2*7 + 6*6).  One SPMD program runs on all 8
    cores; cores with fewer graphs chew zero blocks.
  - The normalized aggregation Tx1 = -D^-1/2 A D^-1/2 feat is a dense
    per-graph matmul against the edge-count matrix C (structural, built
    host-side).  C is stored as EXACT fp8e4 small-int counts and streamed
    from HBM ONCE per graph, resident in SBUF across both Chebyshev layers
    (the baseline streamed scaled fp16 blocks twice: 4x the HBM traffic).
  - The degree scalings are factored out of C:  agg = C^T (dinv*feat),
    Tx1 = -dinv[dst] * agg.  The src scale rides the node-major stationary
    tiles (host-prescaled fp8 for layer 1, a fused DVE tensor_scalar after
    the on-chip transposes for layer 2); the dst scale is a host-staged
    -dinv broadcast tile multiplied into the PSUM->SBUF copy
    (scalar_tensor_tensor), replacing the plain copy at zero extra cost.
  - With both aggregation operands in fp8, the matmuls run in DoubleRow
    perf mode (256-deep contraction per pass) at free-dim 512/464, ~1.4-2x
    the fp16 PE rate.  Dense Chebyshev layers, bias+relu, max-pool readout
    and the classifier stay fp16/fp32 on-device.
"""

import sys

if "/opt/trn_rl_repo" not in sys.path:
    sys.path.insert(0, "/opt/trn_rl_repo")

import numpy as np
import ml_dtypes

# ---------------------------------------------------------------- constants
N = 100_000
E = 1_600_000
B = 50
GSIZE = 2000
D = 128  # IN == HID == 128
NCOUT = 10
NCORES = 8
NG = 7  # graph slots per core (50 = 2*7 + 6*6)
NWIN = 16  # src windows of 128
GSTRIDE = NWIN * 128  # 2048
QUADS = [(0, 512), (512, 512), (1024, 512), (1536, 464)]  # dst tiling of 2000
SROW = NWIN * GSIZE  # S cols per slot (quad-major: [q][t][qn])

F8 = ml_dtypes.float8_e4m3


# ---------------------------------------------------------------- host prep
def _preprocess(src, dst):
    """Structural preprocessing: graph->core assignment, degrees, and
    per-graph edge-count blocks [128, 16, 2000] (partition-major windows)."""
    deg = np.bincount(dst, minlength=N)
    dinv = (np.clip(deg.astype(np.float64), 1.0, None) ** -0.5).astype(np.float32)

    order = [0, 2, 1, 3, 4, 5, 6, 7]  # extra graphs land on cores 0 and 2
    slots = [[] for _ in range(NCORES)]
    for g in range(B):
        slots[order[g % NCORES]].append(g)

    g_of_e = dst // GSIZE
    flat = (src - g_of_e * GSIZE) * np.int64(GSIZE) + (dst - g_of_e * GSIZE)
    cblks = []
    for g in range(B):
        m = g_of_e == g
        cnt = np.bincount(flat[m], minlength=GSTRIDE * GSIZE).astype(np.float32)
        # [2048 src, 2000 dst] -> [128 p, 16 t, 2000 d]
        c = cnt.reshape(NWIN, 128, GSIZE).transpose(1, 0, 2)
        cblks.append(c.astype(F8))
    return dict(slots=slots, cblks=cblks, dinv=dinv)


# ---------------------------------------------------------------- program
def _build_program():
    from concourse import bacc, mybir, tile

    f8 = mybir.dt.float8e4
    f16 = mybir.dt.float16
    f32 = mybir.dt.float32
    AL = mybir.AluOpType
    DR = mybir.MatmulPerfMode.DoubleRow

    nc = bacc.Bacc(None, target_bir_lowering=False)

    xg_in = nc.declare_dram_parameter("XG", [128, NG * GSIZE], f16, isOutput=False)
    ynm_in = nc.declare_dram_parameter("YNM8", [128, NG * GSTRIDE], f8, isOutput=False)
    sc_in = nc.declare_dram_parameter("SC8", [128, NG * SROW], f8, isOutput=False)
    ndb_in = nc.declare_dram_parameter("NDB", [128, NG * GSIZE], f16, isOutput=False)
    dsb_in = nc.declare_dram_parameter(
        "DSRCB", [128, NG * GSTRIDE], f16, isOutput=False
    )
    w1a_in = nc.declare_dram_parameter("W1A", [128, 128], f16, isOutput=False)
    w1b_in = nc.declare_dram_parameter("W1B", [128, 128], f16, isOutput=False)
    w2a_in = nc.declare_dram_parameter("W2A", [128, 128], f16, isOutput=False)
    w2b_in = nc.declare_dram_parameter("W2B", [128, 128], f16, isOutput=False)
    b1_in = nc.declare_dram_parameter("B1", [128, 1], f32, isOutput=False)
    b2_in = nc.declare_dram_parameter("B2", [128, 1], f32, isOutput=False)
    wc_in = nc.declare_dram_parameter("WC", [128, NCOUT], f16, isOutput=False)
    bc_in = nc.declare_dram_parameter("BC", [1, NCOUT], f16, isOutput=False)
    ones_in = nc.declare_dram_parameter("ONES1", [1, NG], f16, isOutput=False)
    id_in = nc.declare_dram_parameter("IDENT", [128, 128], f16, isOutput=False)
    out_dram = nc.declare_dram_parameter("OUT", [NG, NCOUT], f32, isOutput=True)

    with tile.TileContext(nc) as tc:
        with (
            tc.tile_pool(name="const", bufs=1) as cpool,
            tc.tile_pool(name="sblk", bufs=2) as sbpool,
            tc.tile_pool(name="gin", bufs=2) as ginpool,
            tc.tile_pool(name="stg", bufs=2) as stgpool,
            tc.tile_pool(name="tx1", bufs=2) as tx1pool,
            tc.tile_pool(name="h1", bufs=2) as h1pool,
            tc.tile_pool(name="h2", bufs=2) as h2pool,
            tc.tile_pool(name="ptr", bufs=2, space="PSUM") as ptrpool,
            tc.tile_pool(name="pwin", bufs=2, space="PSUM") as pwinpool,
            tc.tile_pool(name="pd", bufs=2, space="PSUM") as pdpool,
        ):
            ident = cpool.tile([128, 128], f16, tag="ident")
            w1a = cpool.tile([128, 128], f16, tag="w1a")
            w1b = cpool.tile([128, 128], f16, tag="w1b")
            w2a = cpool.tile([128, 128], f16, tag="w2a")
            w2b = cpool.tile([128, 128], f16, tag="w2b")
            b1t = cpool.tile([128, 1], f32, tag="b1")
            b2t = cpool.tile([128, 1], f32, tag="b2")
            wct = cpool.tile([128, NCOUT], f16, tag="wc")
            bct = cpool.tile([1, NCOUT], f16, tag="bc")
            ones1 = cpool.tile([1, NG], f16, tag="ones1")
            hg = cpool.tile([128, NG], f16, tag="hg")
            outs = cpool.tile([NG, NCOUT], f32, tag="outs")
            dsrcb = cpool.tile([128, NG * GSTRIDE], f16, tag="dsrcb")

            nc.sync.dma_start(out=ident[:], in_=id_in[:])
            nc.sync.dma_start(out=w1a[:], in_=w1a_in[:])
            nc.sync.dma_start(out=w1b[:], in_=w1b_in[:])
            nc.sync.dma_start(out=w2a[:], in_=w2a_in[:])
            nc.sync.dma_start(out=w2b[:], in_=w2b_in[:])
            nc.sync.dma_start(out=b1t[:], in_=b1_in[:])
            nc.sync.dma_start(out=b2t[:], in_=b2_in[:])
            nc.sync.dma_start(out=wct[:], in_=wc_in[:])
            nc.sync.dma_start(out=bct[:], in_=bc_in[:])
            nc.sync.dma_start(out=ones1[:], in_=ones_in[:])

            for s in range(NG):
                # Per-graph inputs, loaded just-in-time (double-buffered so
                # graph s+1 streams while s computes; avoids a startup stall
                # on one huge const load).
                ynm = ginpool.tile([128, NWIN, 128], f8, tag="ynm")
                nc.sync.dma_start(
                    out=ynm[:],
                    in_=ynm_in[:, s * GSTRIDE : (s + 1) * GSTRIDE].rearrange(
                        "p (t f) -> p t f", f=128
                    ),
                )
                # S blocks for this graph: one tile per dst quad, resident
                # across both layers.  Host layout is quad-major so each
                # partition reads contiguous runs; each quad is split into
                # four DMAs to spread across DMA engines.
                sbq = []
                for qi, (qoff, qn) in enumerate(QUADS):
                    sb = sbpool.tile([128, NWIN, qn], f8, tag=f"sb{qi}")
                    c0 = s * SROW + qoff * NWIN
                    quarter = (NWIN // 4) * qn
                    for hh in range(4):
                        nc.sync.dma_start(
                            out=sb[:, hh * (NWIN // 4) : (hh + 1) * (NWIN // 4), :],
                            in_=sc_in[
                                :, c0 + hh * quarter : c0 + (hh + 1) * quarter
                            ].rearrange("p (t d) -> p t d", t=NWIN // 4),
                        )
                    sbq.append(sb)
                xg = ginpool.tile([128, GSIZE], f16, tag="xg")
                ndb = ginpool.tile([128, GSIZE], f16, tag="ndb")
                nc.sync.dma_start(
                    out=xg[:], in_=xg_in[:, s * GSIZE : (s + 1) * GSIZE]
                )
                nc.sync.dma_start(
                    out=ndb[:], in_=ndb_in[:, s * GSIZE : (s + 1) * GSIZE]
                )
                nc.sync.dma_start(
                    out=dsrcb[:, s * GSTRIDE : (s + 1) * GSTRIDE],
                    in_=dsb_in[:, s * GSTRIDE : (s + 1) * GSTRIDE],
                )

                h1 = h1pool.tile([128, GSTRIDE], f16, tag="h1")
                h2 = h2pool.tile([128, GSIZE], f16, tag="h2")

                for layer in range(2):
                    if layer == 0:
                        stg3 = ynm
                    else:
                        # node-major dinv-scaled fp8 copy of h1: PE
                        # transposes batched 4-per-PSUM-bank (one matmul
                        # group, disjoint column ranges), then one fused
                        # DVE scale/cast per batch
                        stg2 = stgpool.tile([128, NWIN, 128], f8, tag="stg2")
                        for j in range(NWIN // 4):
                            ptr4 = ptrpool.tile([128, 512], f32, tag="ptr")
                            for k in range(4):
                                t = j * 4 + k
                                nc.tensor.matmul(
                                    ptr4[:, k * 128 : (k + 1) * 128],
                                    h1[:, t * 128 : (t + 1) * 128],
                                    ident[:],
                                    start=(k == 0),
                                    stop=(k == 3),
                                    skip_group_check=True,
                                )
                            nc.vector.scalar_tensor_tensor(
                                stg2[:, j * 4 : (j + 1) * 4, :],
                                ptr4[:],
                                1.0,
                                dsrcb[
                                    :, s * GSTRIDE + j * 512 : s * GSTRIDE + (j + 1) * 512
                                ],
                                AL.mult,
                                AL.mult,
                            )
                        stg3 = stg2

                    # aggregation: tx1[f, d] = -dinv[d] * sum_s y[s, f] C[s, d]
                    # quad-outer: quad 0 can start as soon as its S DMA lands
                    tx1 = tx1pool.tile([128, GSIZE], f16, tag="tx1")
                    for qi, (qoff, qn) in enumerate(QUADS):
                        pwin = pwinpool.tile([128, 512], f32, tag="pwin")
                        for th in range(NWIN // 2):
                            nc.tensor.matmul(
                                pwin[:, :qn],
                                stg3[:, 2 * th : 2 * th + 2, :],
                                sbq[qi][:, 2 * th : 2 * th + 2, :],
                                start=(th == 0),
                                stop=(th == NWIN // 2 - 1),
                                perf_mode=DR,
                            )
                        nc.vector.scalar_tensor_tensor(
                            tx1[:, qoff : qoff + qn],
                            pwin[:, :qn],
                            1.0,
                            ndb[:, qoff : qoff + qn],
                            AL.mult,
                            AL.mult,
                        )

                    # dense: h = relu([Tx0, Tx1] @ W + b); bias+relu on the
                    # (otherwise idle) scalar engine
                    wa, wb = (w1a, w1b) if layer == 0 else (w2a, w2b)
                    bt = b1t if layer == 0 else b2t
                    for qoff, qn in QUADS:
                        pd = pdpool.tile([128, 512], f32, tag="pd")
                        rhs0 = (
                            xg[:, qoff : qoff + qn]
                            if layer == 0
                            else h1[:, qoff : qoff + qn]
                        )
                        nc.tensor.matmul(
                            pd[:, :qn], wa[:], rhs0, start=True, stop=False
                        )
                        nc.tensor.matmul(
                            pd[:, :qn],
                            wb[:],
                            tx1[:, qoff : qoff + qn],
                            start=False,
                            stop=True,
                        )
                        dst_ap = (
                            h1[:, qoff : qoff + qn]
                            if layer == 0
                            else h2[:, qoff : qoff + qn]
                        )
                        nc.scalar.activation(
                            dst_ap,
                            pd[:, :qn],
                            mybir.ActivationFunctionType.Relu,
                            bias=bt[:],
                            scale=1.0,
                        )
                    if layer == 0:
                        nc.vector.memset(h1[:, GSIZE:GSTRIDE], 0.0)
                    else:
                        nc.vector.tensor_reduce(
                            hg[:, s : s + 1],
                            h2[:, :GSIZE],
                            mybir.AxisListType.X,
                            AL.max,
                        )

            # ---- readout: out = HG^T @ Wc + 1^T @ bc
            pot = pdpool.tile([128, 512], f32, tag="pd")
            po = pot[:NG, :NCOUT]
            nc.tensor.matmul(po, hg[:, :NG], wct[:], start=True, stop=False)
            nc.tensor.matmul(po, ones1[:], bct[:], start=False, stop=True)
            nc.vector.tensor_copy(outs[:], po)
            nc.sync.dma_start(out=out_dram[:], in_=outs[:])

    nc.compile()
    return nc


# ---------------------------------------------------------------- host glue
def _make_core_inputs(x, W1, b1, W2, b2, Wc, bc, pre):
    dinv = pre["dinv"]
    in_maps = []
    for c in range(NCORES):
        xg = np.zeros((128, NG * GSIZE), dtype=np.float16)
        ynm = np.zeros((128, NG * GSTRIDE), dtype=F8)
        sc = np.zeros((128, NG * SROW), dtype=F8)
        ndb = np.zeros((128, NG * GSIZE), dtype=np.float16)
        dsrcb = np.zeros((128, NG * GSTRIDE), dtype=np.float16)
        for s, g in enumerate(pre["slots"][c]):
            xgf = x[g * GSIZE : (g + 1) * GSIZE]  # [2000, 128] f32
            dv = dinv[g * GSIZE : (g + 1) * GSIZE]  # [2000]
            xg[:, s * GSIZE : (s + 1) * GSIZE] = xgf.T.astype(np.float16)
            ndb[:, s * GSIZE : (s + 1) * GSIZE] = np.broadcast_to(
                (-dv).astype(np.float16), (128, GSIZE)
            )
            y = np.zeros((GSTRIDE, 128), dtype=np.float32)
            y[:GSIZE] = dv[:, None] * xgf
            # [2048, 128] -> [128 p, 16 t, 128 f]
            ynm[:, s * GSTRIDE : (s + 1) * GSTRIDE] = (
                y.reshape(NWIN, 128, 128).transpose(1, 0, 2).reshape(128, GSTRIDE)
            ).astype(F8)
            dvp = np.zeros(GSTRIDE, dtype=np.float32)
            dvp[:GSIZE] = dv
            # dsrcb[p, t*128 + f] = dinv[t*128 + p] (node-major, bcast over f)
            dsrcb[:, s * GSTRIDE : (s + 1) * GSTRIDE] = np.broadcast_to(
                dvp.reshape(NWIN, 128).T.astype(np.float16)[:, :, None],
                (128, NWIN, 128),
            ).reshape(128, GSTRIDE)
            cb = pre["cblks"][g]  # [128, 16, 2000] f8
            parts = [
                cb[:, :, qoff : qoff + qn].reshape(128, NWIN * qn)
                for qoff, qn in QUADS
            ]
            sc[:, s * SROW : (s + 1) * SROW] = np.concatenate(parts, axis=1)
        in_maps.append(
            dict(
                XG=xg,
                YNM8=ynm,
                SC8=sc,
                NDB=ndb,
                DSRCB=dsrcb,
                W1A=np.ascontiguousarray(W1[:128]).astype(np.float16),
                W1B=np.ascontiguousarray(W1[128:]).astype(np.float16),
                W2A=np.ascontiguousarray(W2[:128]).astype(np.float16),
                W2B=np.ascontiguousarray(W2[128:]).astype(np.float16),
                B1=b1.reshape(128, 1).astype(np.float32),
                B2=b2.reshape(128, 1).astype(np.float32),
                WC=Wc.astype(np.float16),
                BC=bc.reshape(1, NCOUT).astype(np.float16),
                ONES1=np.ones((1, NG), dtype=np.float16),
                IDENT=np.eye(128, dtype=np.float16),
            )
        )
    return in_maps


_CACHE = {}


def kernel(x, W1, b1, W2, b2, Wc, bc, src, dst, graph_ids, _trace=False):
    from concourse.bass_utils import run_bass_kernel_spmd

    x = np.asarray(x, dtype=np.float32)
    src = np.asarray(src).astype(np.int64)
    dst = np.asarray(dst).astype(np.int64)

    pre = _preprocess(src, dst)
    if "prog" not in _CACHE:
        _CACHE["prog"] = _build_program()
    nc = _CACHE["prog"]

    in_maps = _make_core_inputs(
        x,
        np.asarray(W1, np.float32),
        np.asarray(b1, np.float32),
        np.asarray(W2, np.float32),
        np.asarray(b2, np.float32),
        np.asarray(Wc, np.float32),
        np.asarray(bc, np.float32),
        pre,
    )
    res = run_bass_kernel_spmd(nc, in_maps, list(range(NCORES)), trace=_trace)

    out = np.zeros((B, NCOUT), dtype=np.float32)
    for c in range(NCORES):
        oc = res.results[c]["OUT"]
        for s, g in enumerate(pre["slots"][c]):
            out[g] = oc[s]
    if _trace:
        kernel._last_exec_ns = res.exec_time_ns
    return out
